# revision 48
# baseline (speedup 1.0000x reference)
"""Trainium2 Bass kernel for a pre-norm multi-head attention block.

Problem: x(4,1024,768) -> LN -> QKV (12 heads x 64) -> softmax attention
-> out proj -> +residual.

Sharding: 8 cores = 4 batches x 2 head-groups (tensor parallel over heads).
Each core computes LN(x[b]) and 6 heads of attention, then a row-parallel
partial of the output projection.  The host sums the two partials per batch
(each core also adds 0.5*x + 0.5*proj_bias so the pair-sum reconstructs the
residual and bias exactly).  Inputs/outputs ship as bf16 (f32 accumulation
on-chip); all matmuls run in bf16 with fp32 PSUM.

Design notes (measured on hardware):
- Scores run in 64-row PE-tiling mode: the head pair (A at partitions 0:64,
  B at 64:128) executes on concurrent array tiles T0/T8 (~2x).  Both
  n-halves of one k-chunk land in a single 2-bank PSUM tile, evacuated by
  one [128,1024] exp on ScalarE.
- The scores stream is exp-paced (its two psum slots recycle at ScalarE's
  exp rate), so one 128-mode work closure (qk chunk / v tile / AV group /
  proj tile) is interleaved after each score chunk: the PE stays busy with
  real flops and - critically - the HAM clock gate stays at 8/8, since
  row-tiled matmuls alone do not register as PE activity for the clock
  gate and the PE would otherwise throttle to 1.2 GHz.
- AV uses the full 128-row array with a ones-column appended to V so the
  softmax denominator falls out of the same accumulation.  (Row-tiled AV
  matmuls fault on this toolchain when the rhs comes from ScalarE-written
  SBUF - empirically bisected - so AV stays in full-array mode and the
  exp tiles are the only ScalarE-written matmul inputs.)
- LN: bn_stats on DVE for even tiles, Copy/Square accumulators on ScalarE
  for odd tiles (both functions live in every activation-table set);
  inv-std via ScalarE Sqrt + DVE reciprocal.  All eight Sqrts run before
  the first attention Exp, so exactly two table loads happen (the Ln/Exp
  mix in the original formulation reloaded tables 16x).
- x -> xnT transposes on the PE (identity matmuls); evacuations split
  between ScalarE and DVE never feed row-tiled matmul inputs.
- The first half of pair-0 scores starts while the second half of LN is
  still running (it only needs q/k chunk j0 for tokens 0:512).
"""

import sys

if "/opt/trn_rl_repo" not in sys.path:
    sys.path.insert(0, "/opt/trn_rl_repo")

import numpy as np

B = 4
N = 1024
DIM = 768
NHEAD = 12
DHEAD = 64
SCALE = DHEAD ** -0.5
G = 2                    # tensor-parallel groups
HPG = NHEAD // G         # heads per group = 6
DG = HPG * DHEAD         # feature dim per group = 384
DVH = DHEAD + 1          # v head width incl. ones column = 65
VW = HPG * DVH           # augmented v width = 390
NT = N // 128            # token tiles = 8
NC = DIM // 128          # input feature chunks = 6
NJ = DG // 128           # output feature chunks per group = 3
NPAIR = HPG // 2         # head pairs per group = 3

_PROGRAM = {}
LAST_RESULTS = None


def _install_profile_hook():
    """The agent image's ``antenv`` lacks ``axon_hooks``, which
    ``bass_utils`` needs for NTFF profiling under axon (BASS_TRACE=1).
    Recreate it from the slim ctypes implementation in trn_agent_boot."""
    import types
    if "antenv.axon_hooks" in sys.modules:
        return
    try:
        from trn_agent_boot.trn_boot import _ntff_profile_via_ctypes
        hook = _ntff_profile_via_ctypes("/opt/axon/libaxon_pjrt.so")
    except Exception:
        hook = None
    mod = types.ModuleType("antenv.axon_hooks")
    mod.get_axon_ntff_profile_hook = lambda: hook
    mod.set_axon_ntff_profile_hook = lambda h: None
    sys.modules["antenv.axon_hooks"] = mod
    try:
        import antenv
        antenv.axon_hooks = mod
    except Exception:
        pass


def _build_program(with_qk_bias=False):
    import concourse.bass as bass
    import concourse.tile as tile
    from concourse import mybir, bacc

    f32 = mybir.dt.float32
    bf16 = mybir.dt.bfloat16

    nc = bacc.Bacc(None)

    X = nc.dram_tensor("X", [N, DIM], bf16, kind="ExternalInput")
    RES = nc.dram_tensor("RES", [N, DIM], bf16, kind="ExternalInput")
    WQ = nc.dram_tensor("WQ", [128, NC, DG], bf16, kind="ExternalInput")
    WK = nc.dram_tensor("WK", [128, NC, DG], bf16, kind="ExternalInput")
    WVA = nc.dram_tensor("WVA", [128, NC, VW], bf16, kind="ExternalInput")
    WPT = nc.dram_tensor("WPT", [128, NJ, DIM], bf16, kind="ExternalInput")
    # [q_bias(384) | k_bias*SCALE(384) | v_bias_aug(390, 1.0 at ones cols) | ones(512)]
    QKVB = nc.dram_tensor("QKVB", [1, 2 * DG + VW + 512], bf16, kind="ExternalInput")
    OUT = nc.dram_tensor("OUT", [N, DIM], bf16, kind="ExternalOutput")

    ONES_OFF = 2 * DG + VW

    Exp = mybir.ActivationFunctionType.Exp
    Sqrt = mybir.ActivationFunctionType.Sqrt
    sub = mybir.AluOpType.subtract
    mult = mybir.AluOpType.mult
    from concourse.masks import make_identity

    with tile.TileContext(nc) as tc:
        with (
            tc.tile_pool(name="consts", bufs=1) as consts,
            tc.tile_pool(name="xin", bufs=8) as xin_p,
            tc.tile_pool(name="stats", bufs=4) as stats_p,
            tc.tile_pool(name="xn", bufs=3) as xn_p,
            tc.tile_pool(name="big", bufs=1) as big_p,
            tc.tile_pool(name="sm", bufs=4) as sm_p,
            tc.tile_pool(name="resp", bufs=8) as res_p,
            tc.tile_pool(name="outp", bufs=2) as out_p,
            tc.tile_pool(name="partp", bufs=8) as part_p,
            tc.tile_pool(name="psav", bufs=3, space="PSUM") as ps_av,
            tc.tile_pool(name="psbig", bufs=2, space="PSUM") as ps_big,
            tc.tile_pool(name="pswarm", bufs=1, space="PSUM") as ps_warm,
        ):
            wq_t = consts.tile([128, NC, DG], bf16, tag="wq")
            wk_t = consts.tile([128, NC, DG], bf16, tag="wk")
            wva_t = consts.tile([128, NC, VW], bf16, tag="wva")
            wpt_t = consts.tile([128, NJ, DIM], bf16, tag="wpt")
            qkvb_t = consts.tile([1, 2 * DG + VW + 512], bf16, tag="qkvb")
            nc.sync.dma_start(qkvb_t[:], QKVB[:])
            ones = qkvb_t[0:1, ONES_OFF:ONES_OFF + 512]

            # warm-up stationary (no DMA dependency); doubles as the
            # transpose identity
            ident = consts.tile([128, 128], bf16, tag="ident")
            make_identity(nc, ident[:])

            xnT = big_p.tile([128, NC, N], bf16, tag="xnT")
            qT = big_p.tile([128, NJ, N], bf16, tag="qT")
            kT = big_p.tile([128, NJ, N], bf16, tag="kT")
            vaug = big_p.tile([128, NT, VW], bf16, tag="vaug")
            aoT = big_p.tile([128, NJ, N], bf16, tag="aoT")
            # double-buffered exp tiles: set s = pair % 2
            eAs = [big_p.tile([128, NT, N], bf16, tag=f"eA{s}", name=f"eA{s}")
                   for s in range(2)]
            eBs = [big_p.tile([128, NT, N], bf16, tag=f"eB{s}", name=f"eB{s}")
                   for s in range(2)]

            # dedicated filler psum: repeated overwrites, never read, so the
            # fillers never wait on a pool slot
            warmp = ps_warm.tile([128, 512], f32, tag="warm")

            def keep_warm(k):
                # dependency-free matmuls: keep the HAM clock gate at 8/8
                for _ in range(k):
                    nc.tensor.matmul(warmp[:, 0:128], ident[:], ident[:],
                                     start=True, stop=True)

            # ---------------- LN ----------------
            xts = [None] * NT

            Copy = mybir.ActivationFunctionType.Copy
            Square = mybir.ActivationFunctionType.Square

            invs = [None] * NT
            means = [None] * NT

            def ln_stats(i, xt):
                mean = stats_p.tile([128, 1], f32, tag="mean")
                va = stats_p.tile([128, 1], f32, tag="va")
                if i % 2 == 0:
                    # stats on DVE (bn_stats)
                    st6 = stats_p.tile([128, 2, 6], f32, tag="st6")
                    for s in range(2):
                        nc.vector.bn_stats(st6[:, s, :],
                                           xt[:, s * 384:(s + 1) * 384])
                    mv = stats_p.tile([128, 2], f32, tag="mv")
                    nc.vector.bn_aggr(mv[:], st6[:])
                    nc.vector.tensor_copy(mean[:], mv[:, 0:1])
                    nc.vector.tensor_scalar_mul(va[:], mv[:, 1:2], float(DIM))
                else:
                    # stats on ScalarE accumulators (Copy/Square live in every
                    # activation table set, so no table reloads)
                    scr = stats_p.tile([128, DIM], bf16, tag="scr")
                    s1 = stats_p.tile([128, 1], f32, tag="s1")
                    s2 = stats_p.tile([128, 1], f32, tag="s2")
                    nc.scalar.activation(scr[:], xt[:], Copy, accum_out=s1[:])
                    nc.scalar.activation(scr[:], xt[:], Square, accum_out=s2[:])
                    nc.vector.tensor_scalar_mul(mean[:], s1[:], 1.0 / DIM)
                    vv = stats_p.tile([128, 1], f32, tag="vv")
                    nc.vector.scalar_tensor_tensor(
                        out=vv[:], in0=mean[:], scalar=-1.0, in1=s1[:],
                        op0=mult, op1=mult)
                    nc.vector.tensor_add(va[:], vv[:], s2[:])
                # std = sqrt(var/(DIM-1)); all eight Sqrts run before the
                # first attention exp, so exactly two table loads happen.
                std = stats_p.tile([128, 1], f32, tag="std")
                nc.scalar.activation(std[:], va[:], Sqrt,
                                     scale=1.0 / float(DIM - 1))
                inv = stats_p.tile([128, 1], f32, tag="inv", name=f"inv{i}")
                nc.vector.reciprocal(inv[:], std[:])
                means[i], invs[i] = mean, inv

            def ln_finish(i, xt):
                xn = xn_p.tile([128, DIM], bf16, tag="xn")
                nc.vector.tensor_scalar(xn[:], xt[:], means[i][:], invs[i][:],
                                        op0=sub, op1=mult)
                # transpose via PE (128x128 identity matmuls), DVE evac
                ptr = ps_big.tile([128, NC * 128], bf16, tag="big", name="tp")
                for c in range(NC):
                    nc.tensor.transpose(ptr[:, c * 128:(c + 1) * 128],
                                        xn[:, c * 128:(c + 1) * 128], ident[:])
                nc.vector.tensor_copy(xnT[:, :, i * 128:(i + 1) * 128], ptr[:])

            # ---------------- QKV (128-row mode) ----------------
            def qk_half(j, h, w_t, dst, boff):
                # n-half h of q or k chunk j: 6 accumulating MMs, N=512
                p = ps_av.tile([128, 512], f32, tag="av", name="qk")
                last = not with_qk_bias
                for c in range(NC):
                    nc.tensor.matmul(p[:], w_t[:, c, j * 128:(j + 1) * 128],
                                     xnT[:, c, h * 512:(h + 1) * 512],
                                     start=(c == 0),
                                     stop=(c == NC - 1 and last))
                if with_qk_bias:
                    nc.tensor.matmul(
                        p[:],
                        qkvb_t[0:1, boff + j * 128:boff + (j + 1) * 128],
                        ones, start=False, stop=True)
                nc.vector.tensor_copy(dst[:, j, h * 512:(h + 1) * 512], p[:])

            def v_one(i):
                for k in (i,):
                    p = ps_av.tile([128, 512], f32, tag="av", name="v")
                    for c in range(NC):
                        nc.tensor.matmul(p[:, 0:VW], xnT[:, c, k * 128:(k + 1) * 128],
                                         wva_t[:, c, :], start=(c == 0),
                                         stop=(c == NC - 1 and not with_qk_bias))
                    if with_qk_bias:
                        nc.tensor.matmul(p[:, 0:VW], ones[0:1, 0:128],
                                         qkvb_t[0:1, 2 * DG:2 * DG + VW],
                                         start=False, stop=True)
                    nc.vector.tensor_copy(vaug[:, k, :], p[:, 0:VW])
                    if not with_qk_bias:
                        nc.gpsimd.memset(vaug[:, k, DHEAD::DVH], 1.0)

            # ---------------- attention ----------------            # ---------------- attention ----------------
            def scores_kc(t, kc, nsel=(0, 1)):
                # one k-chunk of head pair t: row-tiled MMs (T0/T8
                # concurrent) + wide exps
                s = t % 2
                eA, eB = eAs[s], eBs[s]
                psA = ps_big.tile([128, 1024], f32, tag="big", name="sc")
                psB = ps_big.tile([128, 1024], f32, tag="big", name="sc")
                kAc = kT[0:64, t, kc * 128:(kc + 1) * 128]
                kBc = kT[64:128, t, kc * 128:(kc + 1) * 128]
                for n in nsel:
                    ns = slice(n * 512, (n + 1) * 512)
                    nc.tensor.matmul(psA[:, ns], kAc, qT[0:64, t, ns],
                                     start=True, stop=True)
                    nc.tensor.matmul(psB[:, ns], kBc, qT[64:128, t, ns],
                                     start=True, stop=True)
                if nsel == (0, 1):
                    nc.scalar.activation(eA[:, kc, :], psA[:], Exp)
                    nc.scalar.activation(eB[:, kc, :], psB[:], Exp)
                else:
                    for n in nsel:
                        ns = slice(n * 512, (n + 1) * 512)
                        nc.scalar.activation(eA[:, kc, ns], psA[:, ns], Exp)
                        nc.scalar.activation(eB[:, kc, ns], psB[:, ns], Exp)

            def av_norm(t, h, pX, ns):
                hp = (h % 2) * 64
                rs = sm_p.tile([1, 512], f32, tag="rsum")
                nc.vector.tensor_copy(rs[:], pX[64:65, :])
                rc = sm_p.tile([1, 512], f32, tag="recip")
                nc.vector.reciprocal_approx_fast(rc[:], rs[:])
                bc = sm_p.tile([64, 512], f32, tag="bcast")
                nc.gpsimd.partition_broadcast(bc[:], rc[:])
                nc.vector.tensor_mul(aoT[hp:hp + 64, t, ns], pX[0:64, :], bc[:])

            def av_chunks(t):
                # AV for pair t as 8 closures of 4 full-array MMs each.
                # NOTE: row-tiled (64-contraction) AV matmuls fault on HW when
                # the rhs comes from ScalarE-written SBUF (the exp tiles) —
                # empirically verified — so AV uses the full 128-row array.
                s = t % 2

                def group(h, e, n):
                    # one full accumulation group, never interrupted by a
                    # tiling-mode switch
                    ns = slice(n * 512, (n + 1) * 512)
                    pX = ps_av.tile([DVH, 512], f32, tag="av", name="avp")
                    for kc in range(NT):
                        nc.tensor.matmul(pX[:],
                                         vaug[:, kc, h * DVH:(h + 1) * DVH],
                                         e[:, kc, ns],
                                         start=(kc == 0), stop=(kc == NT - 1))
                    av_norm(t, h, pX, ns)

                hA, hB = 2 * t, 2 * t + 1
                out = []
                for n in range(2):
                    for h, e in ((hA, eAs[s]), (hB, eBs[s])):
                        out.append(lambda h=h, e=e, n=n: group(h, e, n))
                return out

            # ---------------- output projection (128-row mode) ----------------
            parts = [None] * NT
            rests = [None] * NT

            def proj_pass(j, i, pool=None):
                pool = pool or ps_av
                pp0 = pool.tile([128, 512], f32, tag=pool is ps_av and "av" or "big",
                                name="pj0")
                pp1 = pool.tile([128, 256], f32, tag=pool is ps_av and "av" or "big",
                                name="pj1")
                if j == 0:
                    lhs = aoT[:, 0, i * 128:(i + 1) * 128]
                    nc.tensor.matmul(pp0[:], lhs, wpt_t[:, 0, 0:512],
                                     start=True, stop=True)
                    nc.tensor.matmul(pp1[:], lhs, wpt_t[:, 0, 512:768],
                                     start=True, stop=True)
                    rt = rests[i]
                    pt = part_p.tile([128, DIM], bf16, tag="part")
                    nc.vector.tensor_add(pt[:, 0:512], pp0[:], rt[:, 0:512])
                    nc.vector.tensor_add(pt[:, 512:768], pp1[:], rt[:, 512:768])
                    parts[i] = pt
                else:
                    # j1 and j2 accumulate in one psum pair
                    for jj in (1, 2):
                        lhs = aoT[:, jj, i * 128:(i + 1) * 128]
                        nc.tensor.matmul(pp0[:], lhs, wpt_t[:, jj, 0:512],
                                         start=(jj == 1), stop=(jj == 2))
                        nc.tensor.matmul(pp1[:], lhs, wpt_t[:, jj, 512:768],
                                         start=(jj == 1), stop=(jj == 2))
                    ot = out_p.tile([128, DIM], bf16, tag="out")
                    nc.vector.tensor_add(ot[:, 0:512], pp0[:],
                                         parts[i][:, 0:512])
                    nc.vector.tensor_add(ot[:, 512:768], pp1[:],
                                         parts[i][:, 512:768])
                    nc.sync.dma_start(OUT[i * 128:(i + 1) * 128, :], ot[:])

            # ---------------- pipeline emission ----------------
            # X tile loads go on the queue first so LN starts immediately;
            # weights follow (needed only once qk/v/proj begin).
            def load_x(i):
                xt = xin_p.tile([128, DIM], bf16, tag="xin", name=f"xt{i}")
                nc.sync.dma_start(xt[:], X[i * 128:(i + 1) * 128, :])
                xts[i] = xt
            for i in range(3):
                load_x(i)
            nc.sync.dma_start(wq_t[:], WQ[:])
            nc.sync.dma_start(wk_t[:], WK[:])
            for i in range(3, NT):
                load_x(i)
            nc.sync.dma_start(wva_t[:], WVA[:])
            nc.sync.dma_start(wpt_t[:], WPT[:])
            nc.sync.dma_start(qkvb_t[:], QKVB[:])

            # P1: LN for the first four tiles goes first so qk j0 can
            # start as early as possible; tiles 4-7 follow, still with every
            # ScalarE Sqrt ahead of the first exp (two table loads total).
            for i in range(4):
                ln_stats(i, xts[i])
                keep_warm(3)
            for i in range(4):
                ln_finish(i, xts[i])
                keep_warm(2)
            # qk j0 n-half 0, then the first half of pair-0 scores can start
            # while the second LN half finishes
            qk_half(0, 0, wq_t, qT, 0)
            qk_half(0, 0, wk_t, kT, DG)
            for i in range(4, NT):
                ln_stats(i, xts[i])
            for i in range(4, NT):
                ln_finish(i, xts[i])
                scores_kc(0, i - 4, nsel=(0,))
            qk_half(0, 1, wq_t, qT, 0)
            qk_half(0, 1, wk_t, kT, DG)

            # The scores for pair t are exp-paced on ScalarE (the two psum
            # slots recycle at the exp rate).  Interleave 128-mode work
            # closures between score chunks: the PE stays busy with real
            # flops AND the HAM clock gate stays warm (row-tiled matmuls
            # alone do not count as PE activity for the clock gate).
            def interleave(slots, work):
                w0 = 0
                for idx, (t, kc, nsel) in enumerate(slots):
                    scores_kc(t, kc, nsel)
                    w1 = (len(work) * (idx + 1) + len(slots) - 1) // len(slots)
                    for w in range(w0, min(w1, len(work))):
                        work[w]()
                    w0 = w1
                for w in range(w0, len(work)):
                    work[w]()

            # ST1: rest of scores p0 // qk j1 + v tiles
            slots1 = [(0, kc, (1,)) for kc in range(4)]
            slots1 += [(0, kc, (0, 1)) for kc in range(4, NT)]
            work1 = [lambda h=h: qk_half(1, h, wq_t, qT, 0) for h in range(2)]
            work1 += [lambda h=h: qk_half(1, h, wk_t, kT, DG) for h in range(2)]
            work1 += [lambda i=i: v_one(i) for i in range(NT)]
            interleave(slots1, work1)
            for i in range(NT):
                rt = res_p.tile([128, DIM], bf16, tag="res", name=f"rt{i}")
                nc.sync.dma_start(rt[:], RES[i * 128:(i + 1) * 128, :])
                rests[i] = rt
            # ST2: scores p1 // qk j2 + AV p0
            slots2 = [(1, kc, (0, 1)) for kc in range(NT)]
            work2 = [lambda h=h: qk_half(2, h, wq_t, qT, 0) for h in range(2)]
            work2 += [lambda h=h: qk_half(2, h, wk_t, kT, DG) for h in range(2)]
            work2 += av_chunks(0)
            interleave(slots2, work2)
            # ST3: scores p2 // AV p1 + proj j0
            slots3 = [(2, kc, (0, 1)) for kc in range(NT)]
            work3 = [lambda i=i: proj_pass(0, i) for i in range(2)]
            work3 += av_chunks(1)
            work3 += [lambda i=i: proj_pass(0, i) for i in range(2, NT)]
            interleave(slots3, work3)
            # ST4 (dense): AV p2 interleaved with proj j1, then j2 + output
            avc = av_chunks(2)
            for ci, ch in enumerate(avc):
                ch()
                keep_warm(2)
            for i in range(NT):
                proj_pass(1, i, pool=ps_big)
                keep_warm(2)

    nc.compile()
    return nc


def _get_program(with_qk_bias=False):
    if with_qk_bias not in _PROGRAM:
        _PROGRAM[with_qk_bias] = _build_program(with_qk_bias)
    return _PROGRAM[with_qk_bias]


def _prep_core_inputs(x_b, q_weight, k_weight, v_weight, q_bias, k_bias,
                      v_bias, g, bf16):
    f = np.float32
    sl = slice(g * DG, (g + 1) * DG)

    def chunked(wt, width, nchunks):
        # (768, width) -> (128, nchunks, width)
        return np.ascontiguousarray(
            wt.reshape(nchunks, 128, width).transpose(1, 0, 2)).astype(bf16)

    wq = chunked(np.ascontiguousarray(q_weight[sl, :].T, dtype=f), DG, NC)
    wk = chunked(np.ascontiguousarray((k_weight[sl, :] * SCALE).T, dtype=f), DG, NC)

    wv = np.ascontiguousarray(v_weight[sl, :].T, dtype=f)          # (768, 384)
    wva = np.zeros((DIM, VW), dtype=f)
    vba = np.zeros((VW,), dtype=f)
    for h in range(HPG):
        wva[:, h * DVH:h * DVH + DHEAD] = wv[:, h * DHEAD:(h + 1) * DHEAD]
        vba[h * DVH:h * DVH + DHEAD] = v_bias[sl][h * DHEAD:(h + 1) * DHEAD]
        vba[h * DVH + DHEAD] = 1.0
    wva = chunked(wva, VW, NC)

    qkvb = np.concatenate([
        q_bias[sl].astype(f), (k_bias[sl] * SCALE).astype(f), vba,
        np.ones((512,), dtype=f)])[None, :].astype(bf16)

    return {
        "X": np.ascontiguousarray(x_b).astype(bf16),
        "WQ": wq, "WK": wk, "WVA": wva,
        "QKVB": np.ascontiguousarray(qkvb),
    }


def kernel(x, q_weight, k_weight, v_weight, q_bias, k_bias, v_bias,
           proj_weight, proj_bias, **_ignored):
    global LAST_RESULTS
    _install_profile_hook()
    import ml_dtypes
    from concourse.bass_utils import run_bass_kernel_spmd

    bf16 = ml_dtypes.bfloat16
    x = np.asarray(x, dtype=np.float32)
    q_weight = np.asarray(q_weight, dtype=np.float32)
    k_weight = np.asarray(k_weight, dtype=np.float32)
    v_weight = np.asarray(v_weight, dtype=np.float32)
    q_bias = np.asarray(q_bias, dtype=np.float32)
    k_bias = np.asarray(k_bias, dtype=np.float32)
    v_bias = np.asarray(v_bias, dtype=np.float32)
    proj_weight = np.asarray(proj_weight, dtype=np.float32)
    proj_bias = np.asarray(proj_bias, dtype=np.float32)

    with_qk_bias = bool(np.any(q_bias) or np.any(k_bias))
    nc = _get_program(with_qk_bias)

    wptT = proj_weight.T  # (din 768, dout 768)
    in_maps = []
    for b in range(B):
        res = (0.5 * x[b] + 0.5 * proj_bias[None, :]).astype(bf16)
        for g in range(G):
            m = _prep_core_inputs(x[b], q_weight, k_weight, v_weight,
                                  q_bias, k_bias, v_bias, g, bf16)
            wpt_g = np.ascontiguousarray(wptT[g * DG:(g + 1) * DG, :],
                                         dtype=np.float32)  # (384, 768)
            m["WPT"] = np.ascontiguousarray(
                wpt_g.reshape(NJ, 128, DIM).transpose(1, 0, 2)).astype(bf16)
            m["RES"] = res
            in_maps.append(m)

    LAST_RESULTS = run_bass_kernel_spmd(nc, in_maps, core_ids=list(range(8)))
    outs = [np.asarray(LAST_RESULTS.results[c]["OUT"], dtype=np.float32)
            for c in range(8)]
    full = np.stack([outs[2 * b] + outs[2 * b + 1] for b in range(B)], axis=0)
    return full.astype(np.float32)


# revision 49
# speedup vs baseline: 1.1876x; 1.1876x over previous
"""Trainium2 Bass kernel for a pre-norm multi-head attention block.

Problem: x(4,1024,768) -> LN -> QKV (12 heads x 64) -> softmax attention
-> out proj -> +residual.

Sharding: 8 cores = 4 batches x 2 head-groups (tensor parallel over heads).
Each core computes LN(x[b]) and 6 heads of attention, then a row-parallel
partial of the output projection.  The host sums the two partials per batch
(each core also adds 0.5*x + 0.5*proj_bias so the pair-sum reconstructs the
residual and bias exactly).  Inputs/outputs ship as bf16 (f32 accumulation
on-chip); all matmuls run in bf16 with fp32 PSUM.

Design notes (measured on hardware):
- Scores run in 64-row PE-tiling mode: the head pair (A at partitions 0:64,
  B at 64:128) executes on concurrent array tiles T0/T8 (~2x).  Both
  n-halves of one k-chunk land in a single 2-bank PSUM tile, evacuated by
  one [128,1024] exp on ScalarE.
- The scores stream is exp-paced (its two psum slots recycle at ScalarE's
  exp rate), so one 128-mode work closure (qk chunk / v tile / AV group /
  proj tile) is interleaved after each score chunk: the PE stays busy with
  real flops and - critically - the HAM clock gate stays at 8/8, since
  row-tiled matmuls alone do not register as PE activity for the clock
  gate and the PE would otherwise throttle to 1.2 GHz.
- AV uses the full 128-row array with a ones-column appended to V so the
  softmax denominator falls out of the same accumulation.  (Row-tiled AV
  matmuls fault on this toolchain when the rhs comes from ScalarE-written
  SBUF - empirically bisected - so AV stays in full-array mode and the
  exp tiles are the only ScalarE-written matmul inputs.)
- LN: bn_stats on DVE for even tiles, Copy/Square accumulators on ScalarE
  for odd tiles (both functions live in every activation-table set);
  inv-std via ScalarE Sqrt + DVE reciprocal.  All eight Sqrts run before
  the first attention Exp, so exactly two table loads happen (the Ln/Exp
  mix in the original formulation reloaded tables 16x).
- x -> xnT transposes on the PE (identity matmuls); evacuations split
  between ScalarE and DVE never feed row-tiled matmul inputs.
- The first half of pair-0 scores starts while the second half of LN is
  still running (it only needs q/k chunk j0 for tokens 0:512).
"""

import sys

if "/opt/trn_rl_repo" not in sys.path:
    sys.path.insert(0, "/opt/trn_rl_repo")

import numpy as np

B = 4
N = 1024
DIM = 768
NHEAD = 12
DHEAD = 64
SCALE = DHEAD ** -0.5
G = 2                    # tensor-parallel groups
HPG = NHEAD // G         # heads per group = 6
DG = HPG * DHEAD         # feature dim per group = 384
DVH = DHEAD + 1          # v head width incl. ones column = 65
VW = HPG * DVH           # augmented v width = 390
NT = N // 128            # token tiles = 8
NC = DIM // 128          # input feature chunks = 6
NJ = DG // 128           # output feature chunks per group = 3
NPAIR = HPG // 2         # head pairs per group = 3

_PROGRAM = {}
LAST_RESULTS = None


def _install_profile_hook():
    """The agent image's ``antenv`` lacks ``axon_hooks``, which
    ``bass_utils`` needs for NTFF profiling under axon (BASS_TRACE=1).
    Recreate it from the slim ctypes implementation in trn_agent_boot."""
    import types
    if "antenv.axon_hooks" in sys.modules:
        return
    try:
        from trn_agent_boot.trn_boot import _ntff_profile_via_ctypes
        hook = _ntff_profile_via_ctypes("/opt/axon/libaxon_pjrt.so")
    except Exception:
        hook = None
    mod = types.ModuleType("antenv.axon_hooks")
    mod.get_axon_ntff_profile_hook = lambda: hook
    mod.set_axon_ntff_profile_hook = lambda h: None
    sys.modules["antenv.axon_hooks"] = mod
    try:
        import antenv
        antenv.axon_hooks = mod
    except Exception:
        pass


def _build_program(with_qk_bias=False):
    import concourse.bass as bass
    import concourse.tile as tile
    from concourse import mybir, bacc

    f32 = mybir.dt.float32
    bf16 = mybir.dt.bfloat16

    nc = bacc.Bacc(None)

    X = nc.dram_tensor("X", [N, DIM], bf16, kind="ExternalInput")
    RES = nc.dram_tensor("RES", [N, DIM], bf16, kind="ExternalInput")
    WQ = nc.dram_tensor("WQ", [128, NC, DG], bf16, kind="ExternalInput")
    WK = nc.dram_tensor("WK", [128, NC, DG], bf16, kind="ExternalInput")
    WVA = nc.dram_tensor("WVA", [128, NC, VW], bf16, kind="ExternalInput")
    WPT = nc.dram_tensor("WPT", [128, NJ, DIM], bf16, kind="ExternalInput")
    # [q_bias(384) | k_bias*SCALE(384) | v_bias_aug(390, 1.0 at ones cols) | ones(512)]
    QKVB = nc.dram_tensor("QKVB", [1, 2 * DG + VW + 512], bf16, kind="ExternalInput")
    OUT = nc.dram_tensor("OUT", [N, DIM], bf16, kind="ExternalOutput")

    ONES_OFF = 2 * DG + VW

    Exp = mybir.ActivationFunctionType.Exp
    Sqrt = mybir.ActivationFunctionType.Sqrt
    sub = mybir.AluOpType.subtract
    mult = mybir.AluOpType.mult
    from concourse.masks import make_identity

    with tile.TileContext(nc) as tc:
        with (
            tc.tile_pool(name="consts", bufs=1) as consts,
            tc.tile_pool(name="xin", bufs=8) as xin_p,
            tc.tile_pool(name="stats", bufs=4) as stats_p,
            tc.tile_pool(name="xn", bufs=3) as xn_p,
            tc.tile_pool(name="big", bufs=1) as big_p,
            tc.tile_pool(name="sm", bufs=4) as sm_p,
            tc.tile_pool(name="resp", bufs=8) as res_p,
            tc.tile_pool(name="outp", bufs=2) as out_p,
            tc.tile_pool(name="partp", bufs=8) as part_p,
            tc.tile_pool(name="psav", bufs=3, space="PSUM") as ps_av,
            tc.tile_pool(name="psbig", bufs=2, space="PSUM") as ps_big,
            tc.tile_pool(name="pswarm", bufs=1, space="PSUM") as ps_warm,
        ):
            wq_t = consts.tile([128, NC, DG], bf16, tag="wq")
            wk_t = consts.tile([128, NC, DG], bf16, tag="wk")
            wva_t = consts.tile([128, NC, VW], bf16, tag="wva")
            wpt_t = consts.tile([128, NJ, DIM], bf16, tag="wpt")
            qkvb_t = consts.tile([1, 2 * DG + VW + 512], bf16, tag="qkvb")
            nc.sync.dma_start(qkvb_t[:], QKVB[:])
            ones = qkvb_t[0:1, ONES_OFF:ONES_OFF + 512]

            # warm-up stationary (no DMA dependency); doubles as the
            # transpose identity
            ident = consts.tile([128, 128], bf16, tag="ident")
            make_identity(nc, ident[:])

            xnT = big_p.tile([128, NC, N], bf16, tag="xnT")
            qT = big_p.tile([128, NJ, N], bf16, tag="qT")
            kT = big_p.tile([128, NJ, N], bf16, tag="kT")
            vaug = big_p.tile([128, NT, VW], bf16, tag="vaug")
            aoT = big_p.tile([128, NJ, N], bf16, tag="aoT")
            # double-buffered exp tiles: set s = pair % 2
            eAs = [big_p.tile([128, NT, N], bf16, tag=f"eA{s}", name=f"eA{s}")
                   for s in range(2)]
            eBs = [big_p.tile([128, NT, N], bf16, tag=f"eB{s}", name=f"eB{s}")
                   for s in range(2)]

            # dedicated filler psum: repeated overwrites, never read, so the
            # fillers never wait on a pool slot
            warmp = ps_warm.tile([128, 512], f32, tag="warm")

            def keep_warm(k):
                # dependency-free matmuls: keep the HAM clock gate at 8/8
                for _ in range(k):
                    nc.tensor.matmul(warmp[:, 0:128], ident[:], ident[:],
                                     start=True, stop=True)

            # ---------------- LN ----------------
            xts = [None] * NT

            Copy = mybir.ActivationFunctionType.Copy
            Square = mybir.ActivationFunctionType.Square

            invs = [None] * NT
            means = [None] * NT

            def ln_stats(i, xt):
                mean = stats_p.tile([128, 1], f32, tag="mean")
                va = stats_p.tile([128, 1], f32, tag="va")
                if i % 2 == 0:
                    # stats on DVE (bn_stats)
                    st6 = stats_p.tile([128, 2, 6], f32, tag="st6")
                    for s in range(2):
                        nc.vector.bn_stats(st6[:, s, :],
                                           xt[:, s * 384:(s + 1) * 384])
                    mv = stats_p.tile([128, 2], f32, tag="mv")
                    nc.vector.bn_aggr(mv[:], st6[:])
                    nc.vector.tensor_copy(mean[:], mv[:, 0:1])
                    nc.vector.tensor_scalar_mul(va[:], mv[:, 1:2], float(DIM))
                else:
                    # stats on ScalarE accumulators (Copy/Square live in every
                    # activation table set, so no table reloads)
                    scr = stats_p.tile([128, DIM], bf16, tag="scr")
                    s1 = stats_p.tile([128, 1], f32, tag="s1")
                    s2 = stats_p.tile([128, 1], f32, tag="s2")
                    nc.scalar.activation(scr[:], xt[:], Copy, accum_out=s1[:])
                    nc.scalar.activation(scr[:], xt[:], Square, accum_out=s2[:])
                    nc.vector.tensor_scalar_mul(mean[:], s1[:], 1.0 / DIM)
                    vv = stats_p.tile([128, 1], f32, tag="vv")
                    nc.vector.scalar_tensor_tensor(
                        out=vv[:], in0=mean[:], scalar=-1.0, in1=s1[:],
                        op0=mult, op1=mult)
                    nc.vector.tensor_add(va[:], vv[:], s2[:])
                # std = sqrt(var/(DIM-1)); all eight Sqrts run before the
                # first attention exp, so exactly two table loads happen.
                std = stats_p.tile([128, 1], f32, tag="std")
                nc.scalar.activation(std[:], va[:], Sqrt,
                                     scale=1.0 / float(DIM - 1))
                inv = stats_p.tile([128, 1], f32, tag="inv", name=f"inv{i}")
                nc.vector.reciprocal(inv[:], std[:])
                means[i], invs[i] = mean, inv

            def ln_finish(i, xt):
                xn = xn_p.tile([128, DIM], bf16, tag="xn")
                nc.vector.tensor_scalar(xn[:], xt[:], means[i][:], invs[i][:],
                                        op0=sub, op1=mult)
                # transpose via PE (128x128 identity matmuls), DVE evac
                ptr = ps_big.tile([128, NC * 128], bf16, tag="big", name="tp")
                for c in range(NC):
                    nc.tensor.transpose(ptr[:, c * 128:(c + 1) * 128],
                                        xn[:, c * 128:(c + 1) * 128], ident[:])
                nc.vector.tensor_copy(xnT[:, :, i * 128:(i + 1) * 128], ptr[:])

            # ---------------- QKV (128-row mode) ----------------
            def qk_half(j, h, w_t, dst, boff):
                # n-half h of q or k chunk j: 6 accumulating MMs, N=512
                p = ps_av.tile([128, 512], f32, tag="av", name="qk")
                last = not with_qk_bias
                for c in range(NC):
                    nc.tensor.matmul(p[:], w_t[:, c, j * 128:(j + 1) * 128],
                                     xnT[:, c, h * 512:(h + 1) * 512],
                                     start=(c == 0),
                                     stop=(c == NC - 1 and last))
                if with_qk_bias:
                    nc.tensor.matmul(
                        p[:],
                        qkvb_t[0:1, boff + j * 128:boff + (j + 1) * 128],
                        ones, start=False, stop=True)
                nc.vector.tensor_copy(dst[:, j, h * 512:(h + 1) * 512], p[:])

            def v_one(i):
                for k in (i,):
                    p = ps_av.tile([128, 512], f32, tag="av", name="v")
                    for c in range(NC):
                        nc.tensor.matmul(p[:, 0:VW], xnT[:, c, k * 128:(k + 1) * 128],
                                         wva_t[:, c, :], start=(c == 0),
                                         stop=(c == NC - 1 and not with_qk_bias))
                    if with_qk_bias:
                        nc.tensor.matmul(p[:, 0:VW], ones[0:1, 0:128],
                                         qkvb_t[0:1, 2 * DG:2 * DG + VW],
                                         start=False, stop=True)
                    nc.vector.tensor_copy(vaug[:, k, :], p[:, 0:VW])
                    if not with_qk_bias:
                        nc.gpsimd.memset(vaug[:, k, DHEAD::DVH], 1.0)

            # ---------------- attention ----------------            # ---------------- attention ----------------
            def scores_kc(t, kc, nsel=(0, 1)):
                # one k-chunk of head pair t: row-tiled MMs (T0/T8
                # concurrent) + wide exps
                s = t % 2
                eA, eB = eAs[s], eBs[s]
                psA = ps_big.tile([128, 1024], f32, tag="big", name="sc")
                psB = ps_big.tile([128, 1024], f32, tag="big", name="sc")
                kAc = kT[0:64, t, kc * 128:(kc + 1) * 128]
                kBc = kT[64:128, t, kc * 128:(kc + 1) * 128]
                for n in nsel:
                    ns = slice(n * 512, (n + 1) * 512)
                    nc.tensor.matmul(psA[:, ns], kAc, qT[0:64, t, ns],
                                     start=True, stop=True)
                    nc.tensor.matmul(psB[:, ns], kBc, qT[64:128, t, ns],
                                     start=True, stop=True)
                if nsel == (0, 1):
                    nc.scalar.activation(eA[:, kc, :], psA[:], Exp)
                    nc.scalar.activation(eB[:, kc, :], psB[:], Exp)
                else:
                    for n in nsel:
                        ns = slice(n * 512, (n + 1) * 512)
                        nc.scalar.activation(eA[:, kc, ns], psA[:, ns], Exp)
                        nc.scalar.activation(eB[:, kc, ns], psB[:, ns], Exp)

            def av_norm(t, h, pX, ns):
                hp = (h % 2) * 64
                rs = sm_p.tile([1, 512], f32, tag="rsum")
                nc.vector.tensor_copy(rs[:], pX[64:65, :])
                rc = sm_p.tile([1, 512], f32, tag="recip")
                nc.vector.reciprocal_approx_fast(rc[:], rs[:])
                bc = sm_p.tile([64, 512], f32, tag="bcast")
                nc.gpsimd.partition_broadcast(bc[:], rc[:])
                nc.vector.tensor_mul(aoT[hp:hp + 64, t, ns], pX[0:64, :], bc[:])

            def av_chunks(t):
                # AV for pair t as 8 closures of 4 full-array MMs each.
                # NOTE: row-tiled (64-contraction) AV matmuls fault on HW when
                # the rhs comes from ScalarE-written SBUF (the exp tiles) —
                # empirically verified — so AV uses the full 128-row array.
                s = t % 2

                def group(h, e, n):
                    # one full accumulation group, never interrupted by a
                    # tiling-mode switch
                    ns = slice(n * 512, (n + 1) * 512)
                    pX = ps_av.tile([DVH, 512], f32, tag="av", name="avp")
                    for kc in range(NT):
                        nc.tensor.matmul(pX[:],
                                         vaug[:, kc, h * DVH:(h + 1) * DVH],
                                         e[:, kc, ns],
                                         start=(kc == 0), stop=(kc == NT - 1))
                    av_norm(t, h, pX, ns)

                hA, hB = 2 * t, 2 * t + 1
                out = []
                for n in range(2):
                    for h, e in ((hA, eAs[s]), (hB, eBs[s])):
                        out.append(lambda h=h, e=e, n=n: group(h, e, n))
                return out

            # ---------------- output projection (128-row mode) ----------------
            parts = [None] * NT
            rests = [None] * NT

            def proj_pass(j, i, pool=None):
                pool = pool or ps_av
                pp0 = pool.tile([128, 512], f32, tag=pool is ps_av and "av" or "big",
                                name="pj0")
                pp1 = pool.tile([128, 256], f32, tag=pool is ps_av and "av" or "big",
                                name="pj1")
                if j == 0:
                    lhs = aoT[:, 0, i * 128:(i + 1) * 128]
                    nc.tensor.matmul(pp0[:], lhs, wpt_t[:, 0, 0:512],
                                     start=True, stop=True)
                    nc.tensor.matmul(pp1[:], lhs, wpt_t[:, 0, 512:768],
                                     start=True, stop=True)
                    rt = rests[i]
                    pt = part_p.tile([128, DIM], bf16, tag="part")
                    nc.vector.tensor_add(pt[:, 0:512], pp0[:], rt[:, 0:512])
                    nc.vector.tensor_add(pt[:, 512:768], pp1[:], rt[:, 512:768])
                    parts[i] = pt
                else:
                    # j1 and j2 accumulate in one psum pair
                    for jj in (1, 2):
                        lhs = aoT[:, jj, i * 128:(i + 1) * 128]
                        nc.tensor.matmul(pp0[:], lhs, wpt_t[:, jj, 0:512],
                                         start=(jj == 1), stop=(jj == 2))
                        nc.tensor.matmul(pp1[:], lhs, wpt_t[:, jj, 512:768],
                                         start=(jj == 1), stop=(jj == 2))
                    ot = out_p.tile([128, DIM], bf16, tag="out")
                    nc.vector.tensor_add(ot[:, 0:512], pp0[:],
                                         parts[i][:, 0:512])
                    nc.vector.tensor_add(ot[:, 512:768], pp1[:],
                                         parts[i][:, 512:768])
                    nc.sync.dma_start(OUT[i * 128:(i + 1) * 128, :], ot[:])

            # ---------------- pipeline emission ----------------
            # X tile loads go on the queue first so LN starts immediately;
            # weights follow (needed only once qk/v/proj begin).
            def load_x(i):
                xt = xin_p.tile([128, DIM], bf16, tag="xin", name=f"xt{i}")
                nc.sync.dma_start(xt[:], X[i * 128:(i + 1) * 128, :])
                xts[i] = xt
            for i in range(3):
                load_x(i)
            nc.sync.dma_start(wq_t[:], WQ[:])
            nc.sync.dma_start(wk_t[:], WK[:])
            for i in range(3, NT):
                load_x(i)
            nc.sync.dma_start(wva_t[:], WVA[:])
            nc.sync.dma_start(wpt_t[:], WPT[:])
            nc.sync.dma_start(qkvb_t[:], QKVB[:])

            # P1: LN stats for every tile first (all ScalarE Sqrts precede the
            # first exp -> exactly two activation-table loads), then finishes.
            for i in range(NT):
                ln_stats(i, xts[i])
                keep_warm(3)
            for i in range(4):
                ln_finish(i, xts[i])
                keep_warm(2)
            # qk j0 n-half 0, then the first half of pair-0 scores can start
            # while the second LN half finishes
            qk_half(0, 0, wq_t, qT, 0)
            qk_half(0, 0, wk_t, kT, DG)
            for i in range(4, NT):
                ln_finish(i, xts[i])
                scores_kc(0, i - 4, nsel=(0,))
            qk_half(0, 1, wq_t, qT, 0)
            qk_half(0, 1, wk_t, kT, DG)

            # The scores for pair t are exp-paced on ScalarE (the two psum
            # slots recycle at the exp rate).  Interleave 128-mode work
            # closures between score chunks: the PE stays busy with real
            # flops AND the HAM clock gate stays warm (row-tiled matmuls
            # alone do not count as PE activity for the clock gate).
            def interleave(slots, work):
                w0 = 0
                for idx, (t, kc, nsel) in enumerate(slots):
                    scores_kc(t, kc, nsel)
                    w1 = (len(work) * (idx + 1) + len(slots) - 1) // len(slots)
                    for w in range(w0, min(w1, len(work))):
                        work[w]()
                    w0 = w1
                for w in range(w0, len(work)):
                    work[w]()

            # ST1: rest of scores p0 // qk j1 + v tiles
            slots1 = [(0, kc, (1,)) for kc in range(4)]
            slots1 += [(0, kc, (0, 1)) for kc in range(4, NT)]
            work1 = [lambda h=h: qk_half(1, h, wq_t, qT, 0) for h in range(2)]
            work1 += [lambda h=h: qk_half(1, h, wk_t, kT, DG) for h in range(2)]
            work1 += [lambda i=i: v_one(i) for i in range(NT)]
            interleave(slots1, work1)
            for i in range(NT):
                rt = res_p.tile([128, DIM], bf16, tag="res", name=f"rt{i}")
                nc.sync.dma_start(rt[:], RES[i * 128:(i + 1) * 128, :])
                rests[i] = rt
            # ST2: scores p1 // qk j2 + AV p0
            slots2 = [(1, kc, (0, 1)) for kc in range(NT)]
            work2 = [lambda h=h: qk_half(2, h, wq_t, qT, 0) for h in range(2)]
            work2 += [lambda h=h: qk_half(2, h, wk_t, kT, DG) for h in range(2)]
            work2 += av_chunks(0)
            interleave(slots2, work2)
            # ST3: scores p2 // AV p1 + proj j0
            slots3 = [(2, kc, (0, 1)) for kc in range(NT)]
            work3 = av_chunks(1)
            work3 += [lambda i=i: proj_pass(0, i) for i in range(NT)]
            interleave(slots3, work3)
            # ST4 (dense): AV p2 interleaved with proj j1, then j2 + output
            avc = av_chunks(2)
            for ci, ch in enumerate(avc):
                ch()
                keep_warm(2)
            for i in range(NT):
                proj_pass(1, i, pool=ps_big)
                keep_warm(2)

    nc.compile()
    return nc


def _get_program(with_qk_bias=False):
    if with_qk_bias not in _PROGRAM:
        _PROGRAM[with_qk_bias] = _build_program(with_qk_bias)
    return _PROGRAM[with_qk_bias]


def _prep_core_inputs(x_b, q_weight, k_weight, v_weight, q_bias, k_bias,
                      v_bias, g, bf16):
    f = np.float32
    sl = slice(g * DG, (g + 1) * DG)

    def chunked(wt, width, nchunks):
        # (768, width) -> (128, nchunks, width)
        return np.ascontiguousarray(
            wt.reshape(nchunks, 128, width).transpose(1, 0, 2)).astype(bf16)

    wq = chunked(np.ascontiguousarray(q_weight[sl, :].T, dtype=f), DG, NC)
    wk = chunked(np.ascontiguousarray((k_weight[sl, :] * SCALE).T, dtype=f), DG, NC)

    wv = np.ascontiguousarray(v_weight[sl, :].T, dtype=f)          # (768, 384)
    wva = np.zeros((DIM, VW), dtype=f)
    vba = np.zeros((VW,), dtype=f)
    for h in range(HPG):
        wva[:, h * DVH:h * DVH + DHEAD] = wv[:, h * DHEAD:(h + 1) * DHEAD]
        vba[h * DVH:h * DVH + DHEAD] = v_bias[sl][h * DHEAD:(h + 1) * DHEAD]
        vba[h * DVH + DHEAD] = 1.0
    wva = chunked(wva, VW, NC)

    qkvb = np.concatenate([
        q_bias[sl].astype(f), (k_bias[sl] * SCALE).astype(f), vba,
        np.ones((512,), dtype=f)])[None, :].astype(bf16)

    return {
        "X": np.ascontiguousarray(x_b).astype(bf16),
        "WQ": wq, "WK": wk, "WVA": wva,
        "QKVB": np.ascontiguousarray(qkvb),
    }


def kernel(x, q_weight, k_weight, v_weight, q_bias, k_bias, v_bias,
           proj_weight, proj_bias, **_ignored):
    global LAST_RESULTS
    _install_profile_hook()
    import ml_dtypes
    from concourse.bass_utils import run_bass_kernel_spmd

    bf16 = ml_dtypes.bfloat16
    x = np.asarray(x, dtype=np.float32)
    q_weight = np.asarray(q_weight, dtype=np.float32)
    k_weight = np.asarray(k_weight, dtype=np.float32)
    v_weight = np.asarray(v_weight, dtype=np.float32)
    q_bias = np.asarray(q_bias, dtype=np.float32)
    k_bias = np.asarray(k_bias, dtype=np.float32)
    v_bias = np.asarray(v_bias, dtype=np.float32)
    proj_weight = np.asarray(proj_weight, dtype=np.float32)
    proj_bias = np.asarray(proj_bias, dtype=np.float32)

    with_qk_bias = bool(np.any(q_bias) or np.any(k_bias))
    nc = _get_program(with_qk_bias)

    wptT = proj_weight.T  # (din 768, dout 768)
    in_maps = []
    for b in range(B):
        res = (0.5 * x[b] + 0.5 * proj_bias[None, :]).astype(bf16)
        for g in range(G):
            m = _prep_core_inputs(x[b], q_weight, k_weight, v_weight,
                                  q_bias, k_bias, v_bias, g, bf16)
            wpt_g = np.ascontiguousarray(wptT[g * DG:(g + 1) * DG, :],
                                         dtype=np.float32)  # (384, 768)
            m["WPT"] = np.ascontiguousarray(
                wpt_g.reshape(NJ, 128, DIM).transpose(1, 0, 2)).astype(bf16)
            m["RES"] = res
            in_maps.append(m)

    LAST_RESULTS = run_bass_kernel_spmd(nc, in_maps, core_ids=list(range(8)))
    outs = [np.asarray(LAST_RESULTS.results[c]["OUT"], dtype=np.float32)
            for c in range(8)]
    full = np.stack([outs[2 * b] + outs[2 * b + 1] for b in range(B)], axis=0)
    return full.astype(np.float32)
